# revision 14
# baseline (speedup 1.0000x reference)
"""Gaussian 2x2 splat (DifferentiableSquareSensor) on 8 Trainium2 NeuronCores.

Full inputs in, full 1024x1024 image out.

Math: x,y are uniform in [0,1), so pixel coords land in [512,1024) and with
sigma=0.1 every Gaussian tap except the nearest 2x2 neighborhood is <= e^-50
(~2e-22 relative) -- invisible in fp32.  The splat therefore reduces to a
separable 2x2 deposit with weights  g(t)=exp(-50 t^2), g(1-t)  per axis,
normalized by (gx0+gx1)(gy0+gy1).

Distribution: points are sharded to cores by 64-column x-range of the active
512x512 region, and within a core are bucketed by (32-col strip, 16-row band)
with boundary duplication.  Each core computes its [512, 64] range on-device:
  phase A: bulk fp32 coordinate/weight math (ACT + DVE + Pool)
  phase B: per-128-point-block one-hot placement tiles built with
           broadcast-AP tensor ops (cross-bucket batched), then two PE
           matmuls per block accumulate the 2x2 outer products into PSUM
           [16, 32] per-bucket accumulators.
The host only shards/buckets/pads inputs and reassembles the strips.
"""

import json
import os
import sys

import numpy as np

for _p in ("/opt/trn_rl_repo", "/root/.axon_site/_ro/trn_rl_repo"):
    if os.path.isdir(_p) and _p not in sys.path:
        sys.path.append(_p)

import concourse.bass as bass
import concourse.mybir as mybir
from concourse.bass_utils import run_bass_kernel_spmd
from concourse.tile import TileContext

P = 128
NCORES = 8
SW = 32               # strip width (cols per bucket)
BH = 16               # band height (rows per bucket)
NSTRIP = 64 // SW     # strips per core (2)
NBAND = 512 // BH     # bands per core (32)
NBUCKET = NSTRIP * NBAND          # 64, bucket = s*NBAND + w
XWIN = SW + 2         # 34
YWIN = BH + 2         # 18
NBATCH = 128          # blocks per batched phase-B build group
CA = 512              # phase-A chunk columns (blocks)
F32 = mybir.dt.float32
F16 = mybir.dt.float16


def _split_multiwait(nc):
    """This walrus build rejects >1 sync-wait per instruction; split extras
    into single-wait NoOps placed immediately before on the same engine."""
    orig = nc.to_json_bytes

    def patched():
        js = json.loads(orig().decode())
        for fn in js["functions"]:
            for blk in fn["blocks"]:
                newlist = []
                for inst in blk["instructions"]:
                    si = inst.get("sync_info")
                    ow = (si or {}).get("on_wait") or []
                    if len(ow) > 1:
                        for k, w in enumerate(ow[:-1]):
                            newlist.append({
                                "name": f"{inst['name']}-w{k}",
                                "opcode": "NoOp",
                                "engine": inst["engine"],
                                "ins": [], "outs": [],
                                "sync_info": {"on_wait": [w], "on_update": []},
                                "bass_nofuse": True,
                            })
                        si["on_wait"] = [ow[-1]]
                    newlist.append(inst)
                blk["instructions"] = newlist
        return json.dumps(js).encode()

    nc.to_json_bytes = patched


def _build_module(nbb):
    """Build the SPMD bass module for per-bucket block count nbb (even).
    NB = NBUCKET*nbb total blocks, bucket-major (strip s outer, band w
    inner), each block = 128 points on partitions."""
    NB = NBUCKET * nbb
    HALF = NB // 2        # columns of strip 0 vs strip 1
    nc = bass.Bass("TRN2", target_bir_lowering=False, debug=False,
                   num_devices=NCORES)
    xs_d = nc.dram_tensor("xs", [P, NB], F32, kind="ExternalInput")
    ys_d = nc.dram_tensor("ys", [P, NB], F32, kind="ExternalInput")
    vs_d = nc.dram_tensor("vs", [P, NB], F32, kind="ExternalInput")
    strip_d = nc.dram_tensor("strip", [512, 64], F32, kind="ExternalOutput")

    nchunks = (NB + CA - 1) // CA
    AF = mybir.ActivationFunctionType
    TT = mybir.AluOpType

    with TileContext(nc) as tc:
        with (
            tc.tile_pool(name="persist", bufs=1) as pers,
            tc.tile_pool(name="chunk", bufs=2) as chk,
            tc.tile_pool(name="ftmp", bufs=1) as ftmp,
            tc.tile_pool(name="batch", bufs=2) as bat,
            tc.tile_pool(name="psum", bufs=1, space="PSUM") as psp,
        ):
            # ---- one-time constants ----
            PIDU = pers.tile([P, 1], mybir.dt.uint32)
            nc.gpsimd.dma_start(
                PIDU[:], nc.partition_id_tensor[0:1, 0:1].to_broadcast([P, 1]))
            PIDF = pers.tile([P, 1], F32)
            nc.vector.tensor_copy(PIDF[:], PIDU[:])
            # SCX[s] = 511 + 64*pid + SW*s  (cxp1 = xpix - SCX)
            SCX = [pers.tile([P, 1], F32, name=f"SCX{s}")
                   for s in range(NSTRIP)]
            for s in range(NSTRIP):
                nc.vector.tensor_scalar(out=SCX[s][:], in0=PIDF[:],
                                        scalar1=64.0,
                                        scalar2=511.0 + SW * s,
                                        op0=TT.mult, op1=TT.add)
            B50 = pers.tile([P, 1], F32)
            nc.vector.memset(B50[:], 50.0)
            # RB[p, j] = 511 + BH*band(j)   (ryc = ypix_base - RB)
            RB = pers.tile([P, NB], F32)
            nc.gpsimd.iota(RB[:], pattern=[[0, NSTRIP], [BH, NBAND], [0, nbb]],
                           base=511, channel_multiplier=0,
                           allow_small_or_imprecise_dtypes=True)
            # pair-duplicated iotas: values 0,0,1,1,... so two blocks'
            # one-hots interleave in adjacent fp16 lanes (DVE 2x mode)
            XIOTA = pers.tile([P, 2 * XWIN], F16)
            nc.gpsimd.iota(XIOTA[:], pattern=[[1, XWIN], [0, 2]], base=0,
                           channel_multiplier=0,
                           allow_small_or_imprecise_dtypes=True)
            YIOTA = pers.tile([P, 2 * YWIN], F16)
            nc.gpsimd.iota(YIOTA[:], pattern=[[1, YWIN], [0, 2]], base=0,
                           channel_multiplier=0,
                           allow_small_or_imprecise_dtypes=True)

            # ---- per-point arrays, one tile per phase-A chunk so that
            # phase-B batches only depend on their own chunk (overlap) ----
            def chunk_tiles(nm):
                return [pers.tile([P, min(CA, NB - i * CA)], F16,
                                  name=f"{nm}{i}") for i in range(nchunks)]
            CXP1s = chunk_tiles("CXP1")
            RYCs = chunk_tiles("RYC")
            GY0s = chunk_tiles("GY0")
            A0s = chunk_tiles("A0")
            A1s = chunk_tiles("A1")

            # ---- phase A ----
            for ci in range(nchunks):
                j0 = ci * CA
                C = min(CA, NB - j0)
                sl = slice(j0, j0 + C)
                X = chk.tile([P, CA], F32, name="X")
                Y = chk.tile([P, CA], F32, name="Y")
                V = chk.tile([P, CA], F32, name="V")
                nc.sync.dma_start(X[:, :C], xs_d[:, sl])
                nc.sync.dma_start(Y[:, :C], ys_d[:, sl])
                nc.sync.dma_start(V[:, :C], vs_d[:, sl])

                XP = ftmp.tile([P, CA], F32, name="XP")
                nc.scalar.activation(XP[:, :C], X[:, :C], AF.Copy,
                                     bias=512.0, scale=512.0)
                YP = ftmp.tile([P, CA], F32, name="YP")
                nc.scalar.activation(YP[:, :C], Y[:, :C], AF.Copy,
                                     bias=512.0, scale=512.0)

                # exact floor/frac: xp in [512,1024) has fp32 exponent 9, so
                # masking the low 14 mantissa bits IS floor(xp); frac exact.
                XB = ftmp.tile([P, CA], F32, name="XB")
                nc.vector.tensor_scalar(out=XB[:, :C].bitcast(mybir.dt.int32),
                                        in0=XP[:, :C].bitcast(mybir.dt.int32),
                                        scalar1=-16384, scalar2=None,
                                        op0=TT.bitwise_and)
                YB = ftmp.tile([P, CA], F32, name="YB")
                nc.vector.tensor_scalar(out=YB[:, :C].bitcast(mybir.dt.int32),
                                        in0=YP[:, :C].bitcast(mybir.dt.int32),
                                        scalar1=-16384, scalar2=None,
                                        op0=TT.bitwise_and)
                TX = ftmp.tile([P, CA], F32, name="TX")
                nc.vector.tensor_tensor(out=TX[:, :C], in0=XP[:, :C],
                                        in1=XB[:, :C], op=TT.subtract)
                TY = ftmp.tile([P, CA], F32, name="TY")
                nc.vector.tensor_tensor(out=TY[:, :C], in0=YP[:, :C],
                                        in1=YB[:, :C], op=TT.subtract)
                # only two taps matter per axis, so the normalized weights
                # are sigmoids:  gx0/(gx0+gx1) = sigmoid(50 - 100 tx), and
                # gx1' = 1 - gx0', gy1' = 1 - gy0'.  Fold v into the x pair:
                # a0 = v sigmoid(50-100 tx), a1 = v - a0; y pair stays as
                # gy0' (fp16) with T1 = YC - T0 in phase B.
                GXS = ftmp.tile([P, CA], F32, name="GXS")
                nc.scalar.activation(GXS[:, :C], TX[:, :C], AF.Sigmoid,
                                     bias=B50[:, 0:1], scale=-100.0)
                nc.scalar.activation(GY0s[ci][:, :C], TY[:, :C], AF.Sigmoid,
                                     bias=B50[:, 0:1], scale=-100.0)
                nc.vector.tensor_tensor(out=A0s[ci][:, :C], in0=V[:, :C],
                                        in1=GXS[:, :C], op=TT.mult)
                nc.vector.tensor_tensor(out=A1s[ci][:, :C], in0=V[:, :C],
                                        in1=A0s[ci][:, :C], op=TT.subtract)
                # cxp1 = xpix_base - (511 + 64 pid + SW s);  split at strip
                # boundary column HALF so the subtrahend is per-partition
                for (lo, hi, s) in ((j0, min(j0 + C, HALF), 0),
                                    (max(j0, HALF), j0 + C, 1)):
                    if lo < hi:
                        a, b = lo - j0, hi - j0
                        nc.vector.tensor_scalar(out=CXP1s[ci][:, a:b],
                                                in0=XB[:, a:b],
                                                scalar1=SCX[s][:, 0:1],
                                                scalar2=None, op0=TT.subtract)
                # ryc = ypix_base - (511 + BH band)
                nc.vector.tensor_tensor(out=RYCs[ci][:, :C], in0=YB[:, :C],
                                        in1=RB[:, sl], op=TT.subtract)

            # ---- phase B ----
            # bucket b (= s*NBAND + w) accumulates at PSUM partitions
            # 32*(w%2)+[0,BH), cols 32*(w//2) + 512*s + [0,SW)
            PS = psp.tile([P, 1024], F32)
            batches = []
            for ci in range(nchunks):
                lo = ci * CA
                hi = min(lo + CA, NB)
                j = lo
                while j < hi:
                    n = min(NBATCH, hi - j)
                    batches.append((ci, j, n))
                    j += n

            def pap(tile_ap, off, dims):
                return bass.AP(tile_ap.tensor, tile_ap.offset + off, dims)

            for ci, j0, nbt in batches:
                jl = j0 - ci * CA
                npair = (nbt + 1) // 2
                # paired views: element (q, f, i) = block 2q+i, window pos f
                XC = bat.tile([P, NBATCH * XWIN], F16, name="XC")
                pdim = XC[:].ap[0]
                nc.vector.tensor_tensor(
                    out=pap(XC[:], 0, [pdim, [2 * XWIN, npair], [2, XWIN], [1, 2]]),
                    in0=pap(XIOTA[:], 0, [XIOTA[:].ap[0], [0, npair], [2, XWIN], [1, 2]]),
                    in1=pap(CXP1s[ci][:], jl, [CXP1s[ci][:].ap[0], [2, npair], [0, XWIN], [1, 2]]),
                    op=TT.is_equal)
                YC = bat.tile([P, NBATCH * YWIN], F16, name="YC")
                nc.vector.tensor_tensor(
                    out=pap(YC[:], 0, [YC[:].ap[0], [2 * YWIN, npair], [2, YWIN], [1, 2]]),
                    in0=pap(YIOTA[:], 0, [YIOTA[:].ap[0], [0, npair], [2, YWIN], [1, 2]]),
                    in1=pap(RYCs[ci][:], jl, [RYCs[ci][:].ap[0], [2, npair], [0, YWIN], [1, 2]]),
                    op=TT.is_equal)
                T0 = bat.tile([P, NBATCH * YWIN], F16, name="T0")
                nc.vector.tensor_tensor(
                    out=pap(T0[:], 0, [T0[:].ap[0], [2 * YWIN, npair], [2, YWIN], [1, 2]]),
                    in0=pap(YC[:], 0, [YC[:].ap[0], [2 * YWIN, npair], [2, YWIN], [1, 2]]),
                    in1=pap(GY0s[ci][:], jl, [GY0s[ci][:].ap[0], [2, npair], [0, YWIN], [1, 2]]),
                    op=TT.mult)
                T1 = bat.tile([P, NBATCH * YWIN], F16, name="T1")
                nc.gpsimd.tensor_tensor(
                    out=pap(T1[:], 0, [T1[:].ap[0], [2 * YWIN, npair], [2, YWIN], [1, 2]]),
                    in0=pap(YC[:], 0, [YC[:].ap[0], [2 * YWIN, npair], [2, YWIN], [1, 2]]),
                    in1=pap(T0[:], 0, [T0[:].ap[0], [2 * YWIN, npair], [2, YWIN], [1, 2]]),
                    op=TT.subtract)
                # L[k, (q,r,i)] = gy0*(r+1==ryc) + gy1*(r==ryc)
                L = bat.tile([P, NBATCH * BH], F16, name="L")
                nc.vector.tensor_tensor(
                    out=pap(L[:], 0, [L[:].ap[0], [2 * BH, npair], [2, BH], [1, 2]]),
                    in0=pap(T0[:], 2, [T0[:].ap[0], [2 * YWIN, npair], [2, BH], [1, 2]]),
                    in1=pap(T1[:], 0, [T1[:].ap[0], [2 * YWIN, npair], [2, BH], [1, 2]]),
                    op=TT.add)
                LA0 = bat.tile([P, NBATCH * BH], F16, name="LA0")
                nc.vector.tensor_tensor(
                    out=pap(LA0[:], 0, [LA0[:].ap[0], [2 * BH, npair], [2, BH], [1, 2]]),
                    in0=pap(L[:], 0, [L[:].ap[0], [2 * BH, npair], [2, BH], [1, 2]]),
                    in1=pap(A0s[ci][:], jl, [A0s[ci][:].ap[0], [2, npair], [0, BH], [1, 2]]),
                    op=TT.mult)
                LA1 = bat.tile([P, NBATCH * BH], F16, name="LA1")
                nc.vector.tensor_tensor(
                    out=pap(LA1[:], 0, [LA1[:].ap[0], [2 * BH, npair], [2, BH], [1, 2]]),
                    in0=pap(L[:], 0, [L[:].ap[0], [2 * BH, npair], [2, BH], [1, 2]]),
                    in1=pap(A1s[ci][:], jl, [A1s[ci][:].ap[0], [2, npair], [0, BH], [1, 2]]),
                    op=TT.mult)

                for b in range(nbt):
                    q, i = b // 2, b % 2
                    g = j0 + b          # global block
                    bkt = g // nbb
                    s, w = bkt // NBAND, bkt % NBAND
                    prow = 32 * (w % 2)
                    pcol = 32 * (w // 2) + 512 * s
                    first = (g % nbb) == 0
                    last = (g % nbb) == nbb - 1
                    out_ap = PS[prow:prow + BH, pcol:pcol + SW]
                    lhsT0 = pap(LA0[:], q * 2 * BH + i, [LA0[:].ap[0], [2, BH]])
                    lhsT1 = pap(LA1[:], q * 2 * BH + i, [LA1[:].ap[0], [2, BH]])
                    rhs0 = pap(XC[:], q * 2 * XWIN + i + 2, [XC[:].ap[0], [2, SW]])
                    rhs1 = pap(XC[:], q * 2 * XWIN + i, [XC[:].ap[0], [2, SW]])
                    nc.tensor.matmul(out=out_ap, lhsT=lhsT0, rhs=rhs0,
                                     start=first, stop=False)
                    nc.tensor.matmul(out=out_ap, lhsT=lhsT1, rhs=rhs1,
                                     start=False, stop=last)

            # ---- writeback ----
            # PS[32(w%2)+r, 32(w//2)+512 s+c] -> strip[BH w + r, SW s + c]
            OUT = pers.tile([P, 1024], F32)
            nc.vector.tensor_copy(OUT[0:48, :], PS[0:48, :])
            full = strip_d[0:512, 0:64]
            for k in range(2):
                for s in range(NSTRIP):
                    band_rows = OUT[32 * k:32 * k + BH, :]
                    src = pap(band_rows, 512 * s,
                              [band_rows.ap[0], [32, NBAND // 2], [1, SW]])
                    dst = bass.AP(full.tensor,
                                  full.offset + (BH * k) * 64 + SW * s,
                                  [[64, BH], [2 * BH * 64, NBAND // 2], [1, SW]])
                    nc.sync.dma_start(dst, src)

    _split_multiwait(nc)
    return nc


def _shard(x, y, v):
    """Host sharding: assign each point (+boundary duplicates) to
    (core, bucket) with bucket = strip*NBAND + band; return per-core padded
    [P, NB] arrays and nbb."""
    xp = (x + np.float32(1.0)) * np.float32(512.0)
    yp = (y + np.float32(1.0)) * np.float32(512.0)
    cx = np.floor(xp).astype(np.int32) - 512          # 0..511
    cy = np.floor(yp).astype(np.int32) - 512

    def assign(cx, cy):
        core = np.clip(cx >> 6, 0, NCORES - 1)
        strip = (np.clip(cx, 0, 511) >> 5) & (NSTRIP - 1)
        band = np.clip(cy >> 4, 0, NBAND - 1)
        return core * NBUCKET + strip * NBAND + band

    xdup = ((cx & (SW - 1)) == SW - 1) & (cx != 511)
    ydup = ((cy & (BH - 1)) == BH - 1) & (cy != 511)
    bothdup = xdup & ydup

    idx = np.arange(x.shape[0], dtype=np.int64)
    parts = [
        (idx, assign(cx, cy)),
        (idx[xdup], assign(cx[xdup] + 1, cy[xdup])),
        (idx[ydup], assign(cx[ydup], cy[ydup] + 1)),
        (idx[bothdup], assign(cx[bothdup] + 1, cy[bothdup] + 1)),
    ]
    all_idx = np.concatenate([p[0] for p in parts])
    key = np.concatenate([p[1] for p in parts])

    order = np.argsort(key, kind="stable")
    all_idx = all_idx[order]
    key = key[order]
    counts = np.bincount(key, minlength=NCORES * NBUCKET)
    maxc = int(counts.max())
    nbb = -(-maxc // P)                   # blocks per bucket
    nbb += nbb % 2                        # even, for block pairing
    NB = NBUCKET * nbb
    slot = NB * P

    starts = np.zeros(NCORES * NBUCKET + 1, dtype=np.int64)
    np.cumsum(counts, out=starts[1:])

    per_core = []
    for c in range(NCORES):
        xs = np.full(slot, 0.25, dtype=np.float32)
        ys = np.full(slot, 0.25, dtype=np.float32)
        vs = np.zeros(slot, dtype=np.float32)
        for bkt in range(NBUCKET):
            k = c * NBUCKET + bkt
            seg = all_idx[starts[k]:starts[k + 1]]
            off = bkt * nbb * P
            xs[off:off + seg.size] = x[seg]
            ys[off:off + seg.size] = y[seg]
            vs[off:off + seg.size] = v[seg]
        per_core.append({
            "xs": np.ascontiguousarray(xs.reshape(NB, P).T),
            "ys": np.ascontiguousarray(ys.reshape(NB, P).T),
            "vs": np.ascontiguousarray(vs.reshape(NB, P).T),
        })
    return per_core, nbb


_CACHE = {}


def kernel(x, y, values):
    x = np.asarray(x, dtype=np.float32)
    y = np.asarray(y, dtype=np.float32)
    v = np.asarray(values, dtype=np.float32)

    per_core, nbb = _shard(x, y, v)
    if nbb not in _CACHE:
        _CACHE[nbb] = _build_module(nbb)
    nc = _CACHE[nbb]

    res = run_bass_kernel_spmd(nc, per_core, core_ids=list(range(NCORES)))

    img = np.zeros((1024, 1024), dtype=np.float32)
    for c in range(NCORES):
        img[512:1024, 512 + 64 * c:512 + 64 * (c + 1)] = res.results[c]["strip"]
    return img
